# revision 7
# baseline (speedup 1.0000x reference)
"""NT-Xent (SimCLR) contrastive loss kernel for Trainium2, 8 NeuronCores.

Strategy (data-parallel, per the sharding hint):
  host: z = l2norm(concat(x_i, x_j))  -> [2B, D] = [8192, 256]
  each core c owns a 1024-row stripe of z and computes its
  [1024, 8192] similarity stripe sim = z_stripe @ z.T via TensorE
  (float32r matmuls, K=256 contraction in PSUM), applies
  exp(2*sim) on ScalarE with fused free-dim accumulation
  (row sums -> denominators), computes positive-pair and diagonal
  dot products on VectorE, assembles per-row loss terms
  log(denom_k) - 2*pos_k on device, and host sums the 8 partial
  outputs (the scalar all-reduce) and divides by 2B.
"""

import numpy as np

B = 4096
D = 256
TWO_B = 2 * B
N_CORES = 8
STRIPE = TWO_B // N_CORES  # 1024 rows per core
M_TILES = STRIPE // 128  # 8 partition tiles per stripe
GROUP = 2048  # columns per PSUM group (4 banks)
N_GROUPS = TWO_B // GROUP  # 4
SUB = 512  # matmul free-dim (one PSUM bank of fp32)
SUBS_PER_GROUP = GROUP // SUB  # 4

_COMPILED = {}


def _build_nc(repeat=1):
    import concourse.bass as bass
    import concourse.mybir as mybir
    import concourse.tile as tile
    from concourse import bacc

    f32 = mybir.dt.float32
    f32r = mybir.dt.float32r
    AF = mybir.ActivationFunctionType
    ALU = mybir.AluOpType

    nc = bacc.Bacc(
        "TRN2", target_bir_lowering=False, debug=False, num_devices=N_CORES
    )

    zt_full = nc.dram_tensor("zt_full", [D, TWO_B], f32r, kind="ExternalInput").ap()
    zt_self = nc.dram_tensor("zt_self", [D, STRIPE], f32r, kind="ExternalInput").ap()
    z_self_rows = nc.dram_tensor(
        "z_self_rows", [STRIPE, D], f32, kind="ExternalInput"
    ).ap()
    z_partner_rows = nc.dram_tensor(
        "z_partner_rows", [STRIPE, D], f32, kind="ExternalInput"
    ).ap()
    loss_rows = nc.dram_tensor(
        "loss_rows", [128, M_TILES], f32, kind="ExternalOutput"
    ).ap()

    with tile.TileContext(nc) as tc:
        with (
            tc.tile_pool(name="big", bufs=1) as big,
            tc.tile_pool(name="scratch", bufs=2) as scratch,
            tc.tile_pool(name="small", bufs=1) as small,
            tc.tile_pool(name="ps", bufs=2, space="PSUM") as psp,
        ):
          for _rep in range(repeat):
            # ---- persistent SBUF loads --------------------------------
            # row-major stripe data for pos/diag dot products:
            # rows_tile[p, m*256+d] = z_rows[m*128+p, d]
            self_rows = big.tile([128, M_TILES * D], f32, tag="self_rows")
            nc.sync.dma_start(
                out=self_rows[:].rearrange("p (m d) -> p m d", d=D),
                in_=z_self_rows.rearrange("(m p) d -> p m d", p=128),
            )
            part_rows = big.tile([128, M_TILES * D], f32, tag="part_rows")
            nc.sync.dma_start(
                out=part_rows[:].rearrange("p (m d) -> p m d", d=D),
                in_=z_partner_rows.rearrange("(m p) d -> p m d", p=128),
            )
            # transposed stripe (lhsT operands), split by K-half
            self_t = []
            for h in range(2):
                t = big.tile([128, STRIPE], f32r, tag=f"self_t{h}", name=f"self_t{h}")
                nc.sync.dma_start(out=t[:], in_=zt_self[h * 128 : (h + 1) * 128, :])
                self_t.append(t)
            # full zT, chunked by group for DMA/compute overlap
            full = {}
            for g in range(N_GROUPS):
                for h in range(2):
                    t = big.tile(
                        [128, GROUP], f32r, tag=f"full{h}_{g}", name=f"full{h}_{g}"
                    )
                    nc.sync.dma_start(
                        out=t[:],
                        in_=zt_full[
                            h * 128 : (h + 1) * 128, g * GROUP : (g + 1) * GROUP
                        ],
                    )
                    full[(h, g)] = t

            # ---- pos / diag dot products on VectorE -------------------
            pos_sb = small.tile([128, M_TILES], f32, tag="pos_sb")
            kk_sb = small.tile([128, M_TILES], f32, tag="kk_sb")
            for m in range(M_TILES):
                msl = slice(m * D, (m + 1) * D)
                ttr_out = scratch.tile([128, D], f32, tag="ttr", name=f"ttr_{m}")
                nc.vector.tensor_mul(ttr_out[:], self_rows[:, msl], part_rows[:, msl])
                nc.vector.tensor_reduce(
                    pos_sb[:, m : m + 1],
                    ttr_out[:],
                    axis=mybir.AxisListType.X,
                    op=ALU.add,
                )
                ttr_out2 = scratch.tile([128, D], f32, tag="ttr", name=f"ttrk_{m}")
                nc.vector.tensor_mul(ttr_out2[:], self_rows[:, msl], self_rows[:, msl])
                nc.vector.tensor_reduce(
                    kk_sb[:, m : m + 1],
                    ttr_out2[:],
                    axis=mybir.AxisListType.X,
                    op=ALU.add,
                )
            # exp(2 * sim_kk) — the diagonal term to subtract from row sums
            ekk = small.tile([128, M_TILES], f32, tag="ekk")
            nc.scalar.activation(ekk[:], kk_sb[:], AF.Exp, scale=2.0)

            # ---- the big gram loop ------------------------------------
            # dsum[:, m*N_GROUPS+g] = sum_j exp(2*sim) over group g's cols
            dsum = small.tile([128, M_TILES * N_GROUPS], f32, tag="dsum")
            for g in range(N_GROUPS):
                for m in range(M_TILES):
                    ps = psp.tile(
                        [128, GROUP], f32, tag="ps", name=f"gram_{g}_{m}"
                    )
                    for s in range(SUBS_PER_GROUP):
                        csl = slice(s * SUB, (s + 1) * SUB)
                        nc.tensor.matmul(
                            ps[:, csl],
                            lhsT=self_t[0][:, m * 128 : (m + 1) * 128],
                            rhs=full[(0, g)][:, csl],
                            start=True,
                            stop=False,
                        )
                        nc.tensor.matmul(
                            ps[:, csl],
                            lhsT=self_t[1][:, m * 128 : (m + 1) * 128],
                            rhs=full[(1, g)][:, csl],
                            start=False,
                            stop=True,
                        )
                    esc = scratch.tile([128, GROUP], f32, tag="esc", name=f"esc_{g}_{m}")
                    idx = m * N_GROUPS + g
                    nc.scalar.activation(
                        esc[:],
                        ps[:],
                        AF.Exp,
                        scale=2.0,
                        accum_out=dsum[:, idx : idx + 1],
                    )

            # ---- assemble per-row loss --------------------------------
            denom = small.tile([128, M_TILES], f32, tag="denom")
            nc.vector.tensor_reduce(
                denom[:],
                dsum[:].rearrange("p (m g) -> p m g", g=N_GROUPS),
                axis=mybir.AxisListType.X,
                op=ALU.add,
            )
            nc.vector.tensor_sub(denom[:], denom[:], ekk[:])
            ln_d = small.tile([128, M_TILES], f32, tag="ln_d")
            nc.scalar.activation(ln_d[:], denom[:], AF.Ln)
            loss_t = small.tile([128, M_TILES], f32, tag="loss_t")
            nc.vector.scalar_tensor_tensor(
                out=loss_t[:],
                in0=pos_sb[:],
                scalar=-2.0,
                in1=ln_d[:],
                op0=ALU.mult,
                op1=ALU.add,
            )
            nc.sync.dma_start(out=loss_rows[:], in_=loss_t[:])

    nc.compile()
    return nc


def _get_nc(repeat=1):
    if repeat not in _COMPILED:
        _COMPILED[repeat] = _build_nc(repeat)
    return _COMPILED[repeat]


def _make_in_maps(x_i: np.ndarray, x_j: np.ndarray):
    x = np.concatenate([np.asarray(x_i), np.asarray(x_j)], axis=0).astype(
        np.float32, copy=False
    )
    norms = np.sqrt(np.sum(x.astype(np.float64) ** 2, axis=1))
    norms = np.maximum(norms, 1e-12).astype(np.float32)
    z = (x / norms[:, None]).astype(np.float32)
    zt = np.ascontiguousarray(z.T)  # [D, 2B]

    in_maps = []
    for c in range(N_CORES):
        lo = c * STRIPE
        hi = lo + STRIPE
        plo = (lo + B) % TWO_B
        in_maps.append(
            {
                "zt_full": zt,
                "zt_self": np.ascontiguousarray(zt[:, lo:hi]),
                "z_self_rows": np.ascontiguousarray(z[lo:hi, :]),
                "z_partner_rows": np.ascontiguousarray(z[plo : plo + STRIPE, :]),
            }
        )
    return in_maps


def _run(x_i, x_j, trace=False, repeat=1):
    from concourse.bass_utils import run_bass_kernel_spmd

    nc = _get_nc(repeat)
    in_maps = _make_in_maps(x_i, x_j)
    res = run_bass_kernel_spmd(
        nc, in_maps, core_ids=list(range(N_CORES)), trace=trace
    )
    total = np.float64(0.0)
    for c in range(N_CORES):
        total += res.results[c]["loss_rows"].astype(np.float64).sum()
    loss = np.float32(total / TWO_B)
    return loss, res


def kernel(x_i: np.ndarray, x_j: np.ndarray) -> np.ndarray:
    loss, _ = _run(x_i, x_j, trace=False)
    return np.asarray(loss, dtype=np.float32)


# revision 14
# speedup vs baseline: 336.9627x; 336.9627x over previous
"""NT-Xent (SimCLR) contrastive loss kernel for Trainium2, 8 NeuronCores.

Strategy (data-parallel, per the sharding hint):
  host: z = l2norm(concat(x_i, x_j))  -> [2B, D] = [8192, 256]
  each core c owns a 1024-row stripe of z and computes its
  [1024, 8192] similarity stripe sim = z_stripe @ z.T via TensorE
  (float32r matmuls, K=256 contraction in PSUM), applies
  exp(2*sim) on ScalarE with fused free-dim accumulation
  (row sums -> denominators), computes positive-pair and diagonal
  dot products on VectorE, assembles per-row loss terms
  log(denom_k) - 2*pos_k on device, and host sums the 8 partial
  outputs (the scalar all-reduce) and divides by 2B.
"""

import numpy as np

B = 4096
D = 256
TWO_B = 2 * B
N_CORES = 8
STRIPE = TWO_B // N_CORES  # 1024 rows per core
M_TILES = STRIPE // 128  # 8 partition tiles per stripe
GROUP = 2048  # columns per PSUM group (4 banks)
N_GROUPS = TWO_B // GROUP  # 4
SUB = 512  # matmul free-dim (one PSUM bank of fp32)
SUBS_PER_GROUP = GROUP // SUB  # 4

_COMPILED = {}


TRI_CHUNKS = 17  # super-chunks per core: band c (16-c) + band 15-c (c+1)
TRI_BAND = 512  # rows per band
TRI_MS = 4  # 128-row m-tiles per band


def _build_nc_tri(repeat=1):
    """Triangle variant: each core computes 17 packed [512, 512] blocks of the
    upper triangle of exp(2*sim) (band-pair balanced), emitting per-block
    row sums (DVE) and column sums (PE ones-matmul). Host assembles denom."""
    import concourse.mybir as mybir
    import concourse.tile as tile
    from concourse import bacc

    f32 = mybir.dt.float32
    bf16 = mybir.dt.bfloat16
    AF = mybir.ActivationFunctionType
    ALU = mybir.AluOpType
    NCH = TRI_CHUNKS

    nc = bacc.Bacc(
        "TRN2", target_bir_lowering=False, debug=False, num_devices=N_CORES
    )

    lhst_sel = nc.dram_tensor(
        "lhst_sel", [D, NCH * 512], bf16, kind="ExternalInput"
    ).ap()
    cols_packed = nc.dram_tensor(
        "cols_packed", [D, NCH * 512], bf16, kind="ExternalInput"
    ).ap()
    z_self_rows = nc.dram_tensor(
        "z_self_rows", [2 * TRI_BAND, D], f32, kind="ExternalInput"
    ).ap()
    z_partner_rows = nc.dram_tensor(
        "z_partner_rows", [2 * TRI_BAND, D], f32, kind="ExternalInput"
    ).ap()
    rs_out = nc.dram_tensor(
        "rs_out", [128, NCH * TRI_MS], f32, kind="ExternalOutput"
    ).ap()
    cs_out = nc.dram_tensor("cs_out", [1, NCH * 512], f32, kind="ExternalOutput").ap()
    pos_out = nc.dram_tensor("pos_out", [128, M_TILES], f32, kind="ExternalOutput").ap()
    kk_out = nc.dram_tensor("kk_out", [128, M_TILES], f32, kind="ExternalOutput").ap()

    with tile.TileContext(nc) as tc:
        with (
            tc.tile_pool(name="big", bufs=1) as big,
            tc.tile_pool(name="scratch", bufs=3) as scratch,
            tc.tile_pool(name="small", bufs=1) as small,
            tc.tile_pool(name="ps", bufs=2, space="PSUM") as psp,
        ):
          for _rep in range(repeat):
            # ---- persistent SBUF loads (chunk-grouped for overlap) ----
            self_rows = big.tile([128, M_TILES * D], f32, tag="self_rows")
            nc.sync.dma_start(
                out=self_rows[:].rearrange("p (m d) -> p m d", d=D),
                in_=z_self_rows.rearrange("(m p) d -> p m d", p=128),
            )
            part_rows = big.tile([128, M_TILES * D], f32, tag="part_rows")
            nc.sync.dma_start(
                out=part_rows[:].rearrange("p (m d) -> p m d", d=D),
                in_=z_partner_rows.rearrange("(m p) d -> p m d", p=128),
            )
            lh = []
            co = []
            for h in range(2):
                t = big.tile([128, NCH * 512], bf16, tag=f"lh{h}", name=f"lh{h}")
                lh.append(t)
                t2 = big.tile([128, NCH * 512], bf16, tag=f"co{h}", name=f"co{h}")
                co.append(t2)
            # DMA in chunk groups of 4 so compute can start early
            for g in range((NCH + 3) // 4):
                csl = slice(g * 4 * 512, min(NCH, (g + 1) * 4) * 512)
                for h in range(2):
                    hs = slice(h * 128, (h + 1) * 128)
                    nc.sync.dma_start(out=lh[h][:, csl], in_=lhst_sel[hs, csl])
                    nc.sync.dma_start(out=co[h][:, csl], in_=cols_packed[hs, csl])

            ones_bf = small.tile([128, 1], bf16, tag="ones_bf")
            nc.vector.memset(ones_bf[:], 1.0)

            # ---- pos / diag dot products on VectorE -------------------
            pos_sb = small.tile([128, M_TILES], f32, tag="pos_sb")
            kk_sb = small.tile([128, M_TILES], f32, tag="kk_sb")
            for m in range(M_TILES):
                msl = slice(m * D, (m + 1) * D)
                ttr_out = scratch.tile([128, D], f32, tag="ttr", name=f"ttr_{m}")
                nc.vector.tensor_mul(ttr_out[:], self_rows[:, msl], part_rows[:, msl])
                nc.vector.tensor_reduce(
                    pos_sb[:, m : m + 1],
                    ttr_out[:],
                    axis=mybir.AxisListType.X,
                    op=ALU.add,
                )
                ttr_out2 = scratch.tile([128, D], f32, tag="ttr", name=f"ttrk_{m}")
                nc.vector.tensor_mul(ttr_out2[:], self_rows[:, msl], self_rows[:, msl])
                nc.vector.tensor_reduce(
                    kk_sb[:, m : m + 1],
                    ttr_out2[:],
                    axis=mybir.AxisListType.X,
                    op=ALU.add,
                )
            nc.sync.dma_start(out=pos_out[:], in_=pos_sb[:])
            nc.sync.dma_start(out=kk_out[:], in_=kk_sb[:])

            # ---- triangle gram loop -----------------------------------
            rs_buf = small.tile([128, NCH * TRI_MS], f32, tag="rs_buf")
            cs_buf = small.tile([1, NCH * 512], f32, tag="cs_buf")
            pending_cs = None  # (esc tile, chunk index) awaiting colsum
            for i in range(NCH):
                isl = slice(i * 512, (i + 1) * 512)
                ps = psp.tile([128, 2048], f32, tag="ps", name=f"gram_{i}")
                for ms in range(TRI_MS):
                    osl = slice(ms * 512, (ms + 1) * 512)
                    wsl = slice(i * 512 + ms * 128, i * 512 + (ms + 1) * 128)
                    nc.tensor.matmul(
                        ps[:, osl], lhsT=lh[0][:, wsl], rhs=co[0][:, isl],
                        start=True, stop=False,
                    )
                    nc.tensor.matmul(
                        ps[:, osl], lhsT=lh[1][:, wsl], rhs=co[1][:, isl],
                        start=False, stop=True,
                    )
                # colsum of the PREVIOUS chunk (delayed so psum slots ping-pong)
                if pending_cs is not None:
                    _emit_cs(nc, psp, ones_bf, pending_cs, cs_buf)
                    pending_cs = None
                esc = scratch.tile([128, 2048], bf16, tag="esc", name=f"esc_{i}")
                nc.scalar.activation(esc[:], ps[:], AF.Exp, scale=2.0)
                nc.vector.tensor_reduce(
                    rs_buf[:, i * TRI_MS : (i + 1) * TRI_MS],
                    esc[:].rearrange("p (m s) -> p m s", s=512),
                    axis=mybir.AxisListType.X,
                    op=ALU.add,
                )
                # chunk 0 is always a diagonal block: host never reads its
                # colsum, so skip its PE/DVE work entirely
                pending_cs = (esc, i) if i > 0 else None
            _emit_cs(nc, psp, ones_bf, pending_cs, cs_buf)
            nc.sync.dma_start(out=rs_out[:], in_=rs_buf[:])
            nc.sync.dma_start(
                out=cs_out[0:1, 512:], in_=cs_buf[0:1, 512:]
            )

    nc.compile()
    return nc


def _emit_cs(nc, psp, ones_bf, pending, cs_buf):
    import concourse.mybir as mybir

    if pending is None:
        return
    f32 = mybir.dt.float32
    esc, i = pending
    cs_ps = psp.tile([1, 512], f32, tag="ps", name=f"cs_{i}")
    for ms in range(TRI_MS):
        nc.tensor.matmul(
            cs_ps[0:1, :],
            lhsT=ones_bf[:],
            rhs=esc[:, ms * 512 : (ms + 1) * 512],
            start=(ms == 0),
            stop=(ms == TRI_MS - 1),
        )
    nc.vector.tensor_copy(
        cs_buf[0:1, i * 512 : (i + 1) * 512], cs_ps[0:1, :]
    )


def _build_nc(repeat=1, variant="full"):
    """variant: 'full' | 'noact' (skip exp, zero dsum) | 'dmaonly' (also skip matmuls)"""
    if variant == "tri":
        return _build_nc_tri(repeat)
    import concourse.bass as bass
    import concourse.mybir as mybir
    import concourse.tile as tile
    from concourse import bacc

    f32 = mybir.dt.float32
    f32r = mybir.dt.float32r
    AF = mybir.ActivationFunctionType
    ALU = mybir.AluOpType

    nc = bacc.Bacc(
        "TRN2", target_bir_lowering=False, debug=False, num_devices=N_CORES
    )

    zt_full = nc.dram_tensor("zt_full", [D, TWO_B], f32r, kind="ExternalInput").ap()
    zt_self = nc.dram_tensor("zt_self", [D, STRIPE], f32r, kind="ExternalInput").ap()
    z_self_rows = nc.dram_tensor(
        "z_self_rows", [STRIPE, D], f32, kind="ExternalInput"
    ).ap()
    z_partner_rows = nc.dram_tensor(
        "z_partner_rows", [STRIPE, D], f32, kind="ExternalInput"
    ).ap()
    loss_rows = nc.dram_tensor(
        "loss_rows", [128, M_TILES], f32, kind="ExternalOutput"
    ).ap()

    with tile.TileContext(nc) as tc:
        with (
            tc.tile_pool(name="big", bufs=1) as big,
            tc.tile_pool(name="scratch", bufs=2) as scratch,
            tc.tile_pool(name="small", bufs=1) as small,
            tc.tile_pool(name="ps", bufs=2, space="PSUM") as psp,
        ):
          for _rep in range(repeat):
            # ---- persistent SBUF loads --------------------------------
            # row-major stripe data for pos/diag dot products:
            # rows_tile[p, m*256+d] = z_rows[m*128+p, d]
            self_rows = big.tile([128, M_TILES * D], f32, tag="self_rows")
            nc.sync.dma_start(
                out=self_rows[:].rearrange("p (m d) -> p m d", d=D),
                in_=z_self_rows.rearrange("(m p) d -> p m d", p=128),
            )
            part_rows = big.tile([128, M_TILES * D], f32, tag="part_rows")
            nc.sync.dma_start(
                out=part_rows[:].rearrange("p (m d) -> p m d", d=D),
                in_=z_partner_rows.rearrange("(m p) d -> p m d", p=128),
            )
            # transposed stripe (lhsT operands), split by K-half
            self_t = []
            for h in range(2):
                t = big.tile([128, STRIPE], f32r, tag=f"self_t{h}", name=f"self_t{h}")
                nc.sync.dma_start(out=t[:], in_=zt_self[h * 128 : (h + 1) * 128, :])
                self_t.append(t)
            # full zT, chunked by group for DMA/compute overlap
            full = {}
            for g in range(N_GROUPS):
                for h in range(2):
                    t = big.tile(
                        [128, GROUP], f32r, tag=f"full{h}_{g}", name=f"full{h}_{g}"
                    )
                    nc.sync.dma_start(
                        out=t[:],
                        in_=zt_full[
                            h * 128 : (h + 1) * 128, g * GROUP : (g + 1) * GROUP
                        ],
                    )
                    full[(h, g)] = t

            # ---- pos / diag dot products on VectorE -------------------
            pos_sb = small.tile([128, M_TILES], f32, tag="pos_sb")
            kk_sb = small.tile([128, M_TILES], f32, tag="kk_sb")
            for m in range(M_TILES):
                msl = slice(m * D, (m + 1) * D)
                ttr_out = scratch.tile([128, D], f32, tag="ttr", name=f"ttr_{m}")
                nc.vector.tensor_mul(ttr_out[:], self_rows[:, msl], part_rows[:, msl])
                nc.vector.tensor_reduce(
                    pos_sb[:, m : m + 1],
                    ttr_out[:],
                    axis=mybir.AxisListType.X,
                    op=ALU.add,
                )
                ttr_out2 = scratch.tile([128, D], f32, tag="ttr", name=f"ttrk_{m}")
                nc.vector.tensor_mul(ttr_out2[:], self_rows[:, msl], self_rows[:, msl])
                nc.vector.tensor_reduce(
                    kk_sb[:, m : m + 1],
                    ttr_out2[:],
                    axis=mybir.AxisListType.X,
                    op=ALU.add,
                )
            # exp(2 * sim_kk) — the diagonal term to subtract from row sums
            ekk = small.tile([128, M_TILES], f32, tag="ekk")
            nc.scalar.activation(ekk[:], kk_sb[:], AF.Exp, scale=2.0)

            # ---- the big gram loop ------------------------------------
            # dsum[:, m*N_GROUPS+g] = sum_j exp(2*sim) over group g's cols
            dsum = small.tile([128, M_TILES * N_GROUPS], f32, tag="dsum")
            if variant != "full":
                nc.vector.memset(dsum[:], 1.0)
            for g in range(N_GROUPS):
                for m in range(M_TILES):
                    if variant != "dmaonly":
                        ps = psp.tile(
                            [128, GROUP], f32, tag="ps", name=f"gram_{g}_{m}"
                        )
                        for s in range(SUBS_PER_GROUP):
                            csl = slice(s * SUB, (s + 1) * SUB)
                            nc.tensor.matmul(
                                ps[:, csl],
                                lhsT=self_t[0][:, m * 128 : (m + 1) * 128],
                                rhs=full[(0, g)][:, csl],
                                start=True,
                                stop=False,
                            )
                            nc.tensor.matmul(
                                ps[:, csl],
                                lhsT=self_t[1][:, m * 128 : (m + 1) * 128],
                                rhs=full[(1, g)][:, csl],
                                start=False,
                                stop=True,
                            )
                    if variant == "full":
                        esc = scratch.tile(
                            [128, GROUP], f32, tag="esc", name=f"esc_{g}_{m}"
                        )
                        idx = m * N_GROUPS + g
                        nc.scalar.activation(
                            esc[:],
                            ps[:],
                            AF.Exp,
                            scale=2.0,
                            accum_out=dsum[:, idx : idx + 1],
                        )

            # ---- assemble per-row loss --------------------------------
            denom = small.tile([128, M_TILES], f32, tag="denom")
            nc.vector.tensor_reduce(
                denom[:],
                dsum[:].rearrange("p (m g) -> p m g", g=N_GROUPS),
                axis=mybir.AxisListType.X,
                op=ALU.add,
            )
            nc.vector.tensor_sub(denom[:], denom[:], ekk[:])
            ln_d = small.tile([128, M_TILES], f32, tag="ln_d")
            nc.scalar.activation(ln_d[:], denom[:], AF.Ln)
            loss_t = small.tile([128, M_TILES], f32, tag="loss_t")
            nc.vector.scalar_tensor_tensor(
                out=loss_t[:],
                in0=pos_sb[:],
                scalar=-2.0,
                in1=ln_d[:],
                op0=ALU.mult,
                op1=ALU.add,
            )
            nc.sync.dma_start(out=loss_rows[:], in_=loss_t[:])

    nc.compile()
    return nc


def _get_nc(repeat=1, variant="full"):
    key = (repeat, variant)
    if key not in _COMPILED:
        _COMPILED[key] = _build_nc(repeat, variant)
    return _COMPILED[key]


def _make_in_maps(x_i: np.ndarray, x_j: np.ndarray):
    x = np.concatenate([np.asarray(x_i), np.asarray(x_j)], axis=0).astype(
        np.float32, copy=False
    )
    norms = np.sqrt(np.sum(x.astype(np.float64) ** 2, axis=1))
    norms = np.maximum(norms, 1e-12).astype(np.float32)
    z = (x / norms[:, None]).astype(np.float32)
    zt = np.ascontiguousarray(z.T)  # [D, 2B]

    in_maps = []
    for c in range(N_CORES):
        lo = c * STRIPE
        hi = lo + STRIPE
        plo = (lo + B) % TWO_B
        in_maps.append(
            {
                "zt_full": zt,
                "zt_self": np.ascontiguousarray(zt[:, lo:hi]),
                "z_self_rows": np.ascontiguousarray(z[lo:hi, :]),
                "z_partner_rows": np.ascontiguousarray(z[plo : plo + STRIPE, :]),
            }
        )
    return in_maps


def _normalize(x_i, x_j):
    x = np.concatenate([np.asarray(x_i), np.asarray(x_j)], axis=0).astype(
        np.float32, copy=False
    )
    norms = np.sqrt(np.sum(x.astype(np.float64) ** 2, axis=1))
    norms = np.maximum(norms, 1e-12).astype(np.float32)
    return (x / norms[:, None]).astype(np.float32)


def _tri_chunklist(c):
    """[(band_index, global_col_chunk_t), ...] for core c — 17 entries."""
    a, b = c, 15 - c
    return [(a, t) for t in range(a, 16)] + [(b, t) for t in range(b, 16)]


def _make_in_maps_tri(x_i, x_j):
    import ml_dtypes

    z = _normalize(x_i, x_j)
    zt = np.ascontiguousarray(z.T)  # [D, 2B] fp32
    zt_bf = zt.astype(ml_dtypes.bfloat16)

    in_maps = []
    for c in range(N_CORES):
        chunks = _tri_chunklist(c)
        lhst = np.empty((D, TRI_CHUNKS * 512), dtype=ml_dtypes.bfloat16)
        cols = np.empty((D, TRI_CHUNKS * 512), dtype=ml_dtypes.bfloat16)
        for i, (band, t) in enumerate(chunks):
            lhst[:, i * 512 : (i + 1) * 512] = zt_bf[:, band * 512 : band * 512 + 512]
            cols[:, i * 512 : (i + 1) * 512] = zt_bf[:, t * 512 : t * 512 + 512]
        rows_idx = np.concatenate(
            [np.arange(c * 512, c * 512 + 512),
             np.arange((15 - c) * 512, (15 - c) * 512 + 512)]
        )
        part_idx = (rows_idx + B) % TWO_B
        in_maps.append(
            {
                "lhst_sel": lhst,
                "cols_packed": cols,
                "z_self_rows": np.ascontiguousarray(z[rows_idx]),
                "z_partner_rows": np.ascontiguousarray(z[part_idx]),
            }
        )
    return in_maps


def _assemble_tri(results):
    denom = np.zeros(TWO_B, dtype=np.float64)
    pos = np.zeros(TWO_B, dtype=np.float64)
    kk = np.zeros(TWO_B, dtype=np.float64)
    p_ar = np.arange(128)
    for c in range(N_CORES):
        chunks = _tri_chunklist(c)
        rs = results[c]["rs_out"].astype(np.float64)  # [128, 17*4]
        cs = results[c]["cs_out"].astype(np.float64)[0]  # [17*512]
        diag_is = {0, 16 - c}
        for i, (band, t) in enumerate(chunks):
            for ms in range(TRI_MS):
                rows = band * 512 + ms * 128 + p_ar
                denom[rows] += rs[:, i * TRI_MS + ms]
            if i not in diag_is:
                denom[t * 512 : t * 512 + 512] += cs[i * 512 : (i + 1) * 512]
        rows_idx = np.concatenate(
            [np.arange(c * 512, c * 512 + 512),
             np.arange((15 - c) * 512, (15 - c) * 512 + 512)]
        )
        po = results[c]["pos_out"].astype(np.float64)
        ko = results[c]["kk_out"].astype(np.float64)
        for m in range(M_TILES):
            rows = rows_idx[m * 128 + p_ar]
            pos[rows] = po[:, m]
            kk[rows] = ko[:, m]
    denom -= np.exp(2.0 * kk)
    loss = (np.log(denom) - 2.0 * pos).sum() / TWO_B
    return np.float32(loss)


def _run(x_i, x_j, trace=False, repeat=1, variant="full"):
    from concourse.bass_utils import run_bass_kernel_spmd

    nc = _get_nc(repeat, variant)
    if variant == "tri":
        in_maps = _make_in_maps_tri(x_i, x_j)
    else:
        in_maps = _make_in_maps(x_i, x_j)
    res = run_bass_kernel_spmd(
        nc, in_maps, core_ids=list(range(N_CORES)), trace=trace
    )
    if variant == "tri":
        return _assemble_tri(res.results), res
    total = np.float64(0.0)
    for c in range(N_CORES):
        total += res.results[c]["loss_rows"].astype(np.float64).sum()
    loss = np.float32(total / TWO_B)
    return loss, res


def kernel(x_i: np.ndarray, x_j: np.ndarray) -> np.ndarray:
    loss, _ = _run(x_i, x_j, trace=False)
    return np.asarray(loss, dtype=np.float32)
